# revision 1
# baseline (speedup 1.0000x reference)
"""Causal multi-head attention (B=4, T=2048, D=1024, H=16) on 8 TRN2 NeuronCores.

Sharding: tensor-parallel over heads. Each core owns 2 heads (a contiguous
128-column block of each of W_q / W_k / W_v and a 128-row block of W_out).
Every core consumes the full (transposed) activation matrix xT and produces a
partial output [B*T, D]; the host sums the 8 partials (the "all-reduce").

Per-core device pipeline (all matmuls in float32r — full PE rate, ~1e-4 rel):
  phase A (per batch b): QT2[128,T] = Wq2h.T @ xT_b, KT2 likewise,
           V2[T,128] = xT_b.T @ Wv2h  (stored as Vaug tiles [128,(64|1)x2]
           with a ones column appended per head).
  phase B (per b, per 512-wide query chunk qc): for each 128-wide key tile kt:
           ST_h[k,q] = KT2_h.T @ QT2_h  (scores, transposed layout),
           causal mask via additive -1e30 tile on the diagonal blocks,
           expST = exp(ST/8) on ACT (straight-through softmax, no max-sub),
           OT_h[65,512] += Vaug_h.T @ expST  (row 64 accumulates the softmax
           denominator via the ones column).
  phase C (per b, qc): recip = 1/OT[64,:], broadcast across partitions with a
           rank-1 PE matmul, OTn = OT[0:64]*recip (both heads -> [128,512]),
           out[t,:] partial = OTn.T @ W_out2h, DMA to DRAM.
"""

import sys
import os

if "/opt/trn_rl_repo" not in sys.path:
    sys.path.insert(0, "/opt/trn_rl_repo")

import numpy as np

B, T, D, H, HD = 4, 2048, 1024, 16, 64
NCORES = 8
HPC = H // NCORES          # heads per core = 2
WBLK = HPC * HD            # 128: per-core head-block width
QC = 512                   # query chunk (matmul moving dim)
NQC = T // QC              # 4
KT = 128                   # key tile
NKT = T // KT              # 16
DC = 128                   # contraction chunk of D
NDC = D // DC              # 8

TRACE = False              # test.py sets kernel.TRACE = True for profiling
LAST_EXEC_NS = None
LAST_RESULTS = None

_MAX_WAITS = 1


def _make_tc_class():
    """TileContext patched for this container's walrus build, which rejects
    instructions carrying more than one sync-wait command (CTRL Drain,
    S3_LW ldweights, ...). Excess waits are hoisted onto freshly inserted
    same-engine NOPs placed immediately before the instruction (engine
    queues are in-order, so semantics are preserved)."""
    import concourse.tile as tile
    import concourse.mybir as mybir
    from concourse.vector_clock import VectorClock, ScopedClock

    class TC(tile.TileContext):
        def _drain_and_barrier(self, tick_clock, wait_clock):
            g = tick_clock.global_clock
            n = len(g)
            for proc in range(n):
                t = g[proc]
                if t > 0:
                    nop = self.nc.sync.nop(nofuse=True)
                    vc = VectorClock([0] * n)
                    vc.require_at_least(proc, t)
                    wait_clock.add_sem_waits(nop.ins, ScopedClock({None: vc}))
            self.nc.sync.drain()
            self.nc.all_engine_barrier()
            popped = self.nc._tile_sem_poison_stack.pop()
            assert popped is self._sem_poison
            self.nc.clear_and_free_semaphores(list(self.sems.allocated().values()))
            self.nc.all_engine_barrier()

        def _lower_ordered_insts(self, ordered):
            for bb_name in list(ordered.keys()):
                insts = ordered[bb_name]
                new_insts = []
                for inst in insts:
                    si = inst.sync_info
                    ow = list(si.on_wait) if si is not None and si.on_wait else []
                    if len(ow) > _MAX_WAITS:
                        keep = ow[:_MAX_WAITS]
                        extra = ow[_MAX_WAITS:]
                        for w in extra:
                            nop = mybir.InstNoOp(
                                name=f"WSPL-{self.nc.next_id()}", ins=[], outs=[]
                            )
                            nop.engine = inst.engine
                            nop.bass_nofuse = True
                            nop.sync_info = mybir.SyncInfo(on_wait=[w], on_update=[])
                            new_insts.append(nop)
                        inst.sync_info = mybir.SyncInfo(
                            on_wait=keep,
                            on_update=list(si.on_update) if si.on_update else [],
                        )
                    new_insts.append(inst)
                ordered[bb_name] = new_insts
            return super()._lower_ordered_insts(ordered)

    return TC


def _install_ntff_hook():
    """Provide antenv.axon_hooks (absent from the container's antenv stub) so
    run_bass_kernel_spmd(trace=True) can capture NTFF profiles."""
    import types
    import antenv

    if "antenv.axon_hooks" in sys.modules:
        return
    mod = types.ModuleType("antenv.axon_hooks")
    mod._hook = None
    mod.set_axon_ntff_profile_hook = lambda h: setattr(mod, "_hook", h)
    mod.get_axon_ntff_profile_hook = lambda: mod._hook
    sys.modules["antenv.axon_hooks"] = mod
    antenv.axon_hooks = mod
    try:
        from trn_agent_boot.trn_boot import _ntff_profile_via_ctypes

        hook = _ntff_profile_via_ctypes("/opt/axon/libaxon_pjrt.so")
        if hook is not None:
            mod.set_axon_ntff_profile_hook(hook)
    except Exception as e:  # profiling is best-effort
        print("ntff hook install failed:", e)


def _build_program():
    import concourse.bass as bass
    from concourse import mybir

    TC = _make_tc_class()
    f32 = mybir.dt.float32
    f32r = mybir.dt.float32r
    Exp = mybir.ActivationFunctionType.Exp
    BT = B * T

    nc = bass.Bass("TRN2", target_bir_lowering=False, debug=False, num_devices=NCORES)
    # host-tiled xT: [dc, b, tcq, 128, 512], each [128,512] tile contiguous
    xt_d = nc.dram_tensor("xt", [NDC, B, NQC, DC, QC], f32, kind="ExternalInput")
    wq_d = nc.dram_tensor("wq", [D, WBLK], f32, kind="ExternalInput")
    wk_d = nc.dram_tensor("wk", [D, WBLK], f32, kind="ExternalInput")
    wv_d = nc.dram_tensor("wv", [D, WBLK], f32, kind="ExternalInput")
    wo_d = nc.dram_tensor("wo", [WBLK, D], f32, kind="ExternalInput")
    ones_d = nc.dram_tensor("ones", [128, 64], f32, kind="ExternalInput")
    # host-untiled output: [b, qc, ts, nn2, 128, 512], each tile contiguous
    out_d = nc.dram_tensor("out", [B, NQC, QC // KT, D // QC, 128, QC], f32,
                           kind="ExternalOutput")

    with TC(nc, num_cores=NCORES) as tc:
        from contextlib import ExitStack

        with ExitStack() as ctx:
            constp = ctx.enter_context(tc.tile_pool(name="constp", bufs=1))
            wp = ctx.enter_context(tc.tile_pool(name="wp", bufs=1))
            xtp = ctx.enter_context(tc.tile_pool(name="xtp", bufs=10))
            qkp = ctx.enter_context(tc.tile_pool(name="qkp", bufs=2))
            vap = ctx.enter_context(tc.tile_pool(name="vap", bufs=2))
            expp = ctx.enter_context(tc.tile_pool(name="expp", bufs=3))
            otnp = ctx.enter_context(tc.tile_pool(name="otnp", bufs=2))
            outsbp = ctx.enter_context(tc.tile_pool(name="outsbp", bufs=3))
            rcpp = ctx.enter_context(tc.tile_pool(name="rcpp", bufs=2))
            bcp = ctx.enter_context(tc.tile_pool(name="bcp", bufs=2))
            # PSUM: st(3) + mix(3) + ot0(1) + ot1(1) = 8 banks
            stp = ctx.enter_context(tc.tile_pool(name="stp", bufs=3, space="PSUM"))
            mixp = ctx.enter_context(tc.tile_pool(name="mixp", bufs=3, space="PSUM"))
            otp = ctx.enter_context(tc.tile_pool(name="otp", bufs=1, space="PSUM"))

            # ---- one-time constants / weights ----
            ones64 = constp.tile([1, 64], f32r)
            nc.gpsimd.dma_start(ones64[:], ones_d[0:1, 0:64])
            ident = constp.tile([128, 128], f32)
            from concourse.masks import make_identity
            make_identity(nc, ident[:])
            maskadd = []
            for j in range(QC // KT):  # 4 diagonal offsets
                m = constp.tile([128, QC], f32, name=f"maskadd{j}")
                nc.gpsimd.memset(m[:], 0.0)
                # keep (add 0) where q - k >= 0: q = qc0 + y, k = qc0 + 128j + x
                nc.gpsimd.affine_select(
                    out=m[:],
                    in_=m[:],
                    compare_op=mybir.AluOpType.is_ge,
                    fill=-1e30,
                    base=-KT * j,
                    pattern=[[1, QC]],
                    channel_multiplier=-1,
                )
                maskadd.append(m)

            wq_t = wp.tile([DC, NDC, WBLK], f32r)
            nc.gpsimd.dma_start(wq_t[:], wq_d.rearrange("(c p) m -> p c m", p=DC))
            wk_t = wp.tile([DC, NDC, WBLK], f32r)
            nc.gpsimd.dma_start(wk_t[:], wk_d.rearrange("(c p) m -> p c m", p=DC))
            wv_t = wp.tile([DC, NDC, WBLK], f32r)
            nc.gpsimd.dma_start(wv_t[:], wv_d.rearrange("(c p) m -> p c m", p=DC))
            wo_t = wp.tile([WBLK, D], f32r)
            nc.gpsimd.dma_start(wo_t[:], wo_d[:, :])

            for b in range(B):
                # ---------------- phase A: QKV projection ----------------
                qt2 = qkp.tile([128, T], f32r, tag="qt2", name=f"qt2_b{b}")
                kt2 = qkp.tile([128, T], f32r, tag="kt2", name=f"kt2_b{b}")
                # vaug[:, kt, 0:65] = [V_h0 | 1]; [:, kt, 65:130] = [V_h1 | 1]
                vaug = vap.tile([128, NKT, 130], f32r, tag="vaug", name=f"vaug_b{b}")
                nc.gpsimd.dma_start(
                    vaug[:, :, 64:130:65],
                    ones_d[:, 0:2 * NKT].rearrange("p (a b) -> p a b", b=2),
                )

                for tcq in range(NQC):
                    accq = mixp.tile([128, QC], f32, tag="mix", name=f"accq_b{b}t{tcq}")
                    acck = mixp.tile([128, QC], f32, tag="mix", name=f"acck_b{b}t{tcq}")
                    accv = mixp.tile([128, QC], f32, tag="mix", name=f"accv_b{b}t{tcq}")
                    xts = []
                    for dc in range(NDC):
                        xt_t = xtp.tile([DC, QC], f32r, tag="xt", name=f"xt_b{b}t{tcq}d{dc}")
                        nc.gpsimd.dma_start(xt_t[:], xt_d[dc, b, tcq])
                        xts.append(xt_t)
                    for dc in range(NDC):
                        st, sp = (dc == 0), (dc == NDC - 1)
                        nc.tensor.matmul(
                            accq[:], wq_t[:, dc, :], xts[dc][:],
                            start=st, stop=sp, skip_group_check=True,
                        )
                        nc.tensor.matmul(
                            acck[:], wk_t[:, dc, :], xts[dc][:],
                            start=st, stop=sp, skip_group_check=True,
                        )
                        # V^T [128(2h*64), t]: N=512 streams instead of 4x N=128
                        nc.tensor.matmul(
                            accv[:], wv_t[:, dc, :], xts[dc][:],
                            start=st, stop=sp, skip_group_check=True,
                        )
                    nc.vector.tensor_copy(qt2[:, tcq * QC:(tcq + 1) * QC], accq[:])
                    nc.vector.tensor_copy(kt2[:, tcq * QC:(tcq + 1) * QC], acck[:])
                    # transpose V^T -> V via PE, 128x128 blocks
                    vt_sb = bcp.tile([128, QC], f32, tag="vtsb", name=f"vtsb_b{b}t{tcq}")
                    nc.vector.tensor_copy(vt_sb[:], accv[:])
                    vtr = mixp.tile([128, QC], f32, tag="mix", name=f"vtr_b{b}t{tcq}")
                    for ttl in range(QC // KT):
                        nc.tensor.transpose(
                            vtr[:, ttl * 128:(ttl + 1) * 128],
                            vt_sb[:, ttl * 128:(ttl + 1) * 128],
                            ident[:],
                        )
                    for ttl in range(QC // KT):
                        kt_g = tcq * (QC // KT) + ttl
                        nc.vector.tensor_copy(
                            vaug[:, kt_g, 0:64], vtr[:, ttl * 128: ttl * 128 + 64]
                        )
                        nc.vector.tensor_copy(
                            vaug[:, kt_g, 65:129], vtr[:, ttl * 128 + 64: ttl * 128 + 128]
                        )

                # ------------- phase B + C: attention per query chunk -------------
                for qc in range(NQC):
                    nkt = (qc + 1) * (QC // KT)
                    qsl = slice(qc * QC, (qc + 1) * QC)
                    ot0 = otp.tile([65, QC], f32, tag="ot0", name=f"ot0_b{b}q{qc}")
                    ot1 = otp.tile([65, QC], f32, tag="ot1", name=f"ot1_b{b}q{qc}")
                    pend = None  # (expst0, expst1, kt, w0) awaiting AV matmul
                    for kt in range(nkt):
                        ksl = slice(kt * KT, (kt + 1) * KT)
                        # diagonal tiles: columns left of q = kt*128 are fully
                        # masked - skip them in scores/exp/AV entirely
                        j = kt - qc * (QC // KT)
                        w0 = KT * j if j > 0 else 0
                        csl = slice(w0, QC)
                        qsl_w = slice(qc * QC + w0, (qc + 1) * QC)
                        st0 = stp.tile([128, QC], f32, tag="st", name=f"st0_b{b}q{qc}k{kt}")
                        nc.tensor.matmul(
                            st0[:, csl], kt2[0:64, ksl], qt2[0:64, qsl_w],
                            start=True, stop=True, skip_group_check=True,
                        )
                        st1 = stp.tile([128, QC], f32, tag="st", name=f"st1_b{b}q{qc}k{kt}")
                        nc.tensor.matmul(
                            st1[:, csl], kt2[64:128, ksl], qt2[64:128, qsl_w],
                            start=True, stop=True, skip_group_check=True,
                        )
                        if 0 <= j < 4:
                            msl = slice(0, QC - w0)
                            nc.vector.tensor_add(st0[:, csl], st0[:, csl], maskadd[0][:, msl])
                            nc.vector.tensor_add(st1[:, csl], st1[:, csl], maskadd[0][:, msl])
                        e0 = expp.tile([128, QC], f32r, tag="e0", name=f"e0_b{b}q{qc}k{kt}")
                        nc.scalar.activation(e0[:, csl], st0[:, csl], Exp, scale=0.125)
                        e1 = expp.tile([128, QC], f32r, tag="e1", name=f"e1_b{b}q{qc}k{kt}")
                        nc.scalar.activation(e1[:, csl], st1[:, csl], Exp, scale=0.125)
                        if pend is not None:
                            p0, p1, pk, pw = pend
                            nc.tensor.matmul(
                                ot0[0:65, pw:QC], vaug[:, pk, 0:65], p0[:, pw:QC],
                                start=(pk == 0), stop=False, skip_group_check=True,
                            )
                            nc.tensor.matmul(
                                ot1[0:65, pw:QC], vaug[:, pk, 65:130], p1[:, pw:QC],
                                start=(pk == 0), stop=False, skip_group_check=True,
                            )
                        pend = (e0, e1, kt, w0)
                    p0, p1, pk, pw = pend
                    nc.tensor.matmul(
                        ot0[0:65, pw:QC], vaug[:, pk, 0:65], p0[:, pw:QC],
                        start=(pk == 0), stop=True, skip_group_check=True,
                    )
                    nc.tensor.matmul(
                        ot1[0:65, pw:QC], vaug[:, pk, 65:130], p1[:, pw:QC],
                        start=(pk == 0), stop=True, skip_group_check=True,
                    )

                    # normalize: otn[h*64:(h+1)*64, :] = ot_h[0:64] * (1/denom_h)
                    # (broadcast the raw denominator row across 64 partitions
                    # with a rank-1 PE matmul first, THEN reciprocal on 64
                    # lanes - a [1,512] DVE reciprocal is single-lane, 3.3us)
                    otn = otnp.tile([128, QC], f32r, tag="otn", name=f"otn_b{b}q{qc}")
                    for h, ot in ((0, ot0), (1, ot1)):
                        den = rcpp.tile([1, QC], f32r, tag=f"r{h}", name=f"den_b{b}q{qc}h{h}")
                        nc.vector.tensor_copy(den[:], ot[64:65, :])
                        bc = stp.tile([64, QC], f32, tag="st", name=f"bc_b{b}q{qc}h{h}")
                        nc.tensor.matmul(
                            bc[:], ones64[:], den[:],
                            start=True, stop=True, skip_group_check=True,
                        )
                        bcs = bcp.tile([64, QC], f32, tag="bcs", name=f"bcs_b{b}q{qc}h{h}")
                        nc.vector.reciprocal(bcs[:], bc[:])
                        nc.vector.tensor_mul(
                            otn[h * 64:(h + 1) * 64, :], ot[0:64, :], bcs[:]
                        )

                    # out projection: out[t, :] += OTn.T @ W_out2h
                    for ts in range(QC // KT):
                        for nn2 in range(D // QC):
                            ops = stp.tile(
                                [128, QC], f32, tag="st", name=f"ops_b{b}q{qc}s{ts}n{nn2}"
                            )
                            nc.tensor.matmul(
                                ops[:],
                                otn[:, ts * 128:(ts + 1) * 128],
                                wo_t[:, nn2 * QC:(nn2 + 1) * QC],
                                start=True, stop=True, skip_group_check=True,
                            )
                            osb = outsbp.tile(
                                [128, QC], f32, tag="osb", name=f"osb_b{b}q{qc}s{ts}n{nn2}"
                            )
                            nc.vector.tensor_copy(osb[:], ops[:])
                            nc.sync.dma_start(out_d[b, qc, ts, nn2], osb[:])
    return nc


def kernel(x, W_qkv, W_out):
    global LAST_EXEC_NS, LAST_RESULTS
    from concourse.bass_utils import run_bass_kernel_spmd

    if TRACE:
        _install_ntff_hook()

    x = np.ascontiguousarray(x, dtype=np.float32)
    W_qkv = np.ascontiguousarray(W_qkv, dtype=np.float32)
    W_out = np.ascontiguousarray(W_out, dtype=np.float32)

    xT = x.transpose(2, 0, 1).reshape(D, B * T)
    # tile to [dc, b, tcq, 128, 512] so each device DMA is one contiguous block
    xtt = np.ascontiguousarray(
        xT.reshape(NDC, DC, B, NQC, QC).transpose(0, 2, 3, 1, 4)
    )
    in_maps = []
    for c in range(NCORES):
        cs = slice(c * WBLK, (c + 1) * WBLK)
        in_maps.append({
            "xt": xtt,
            "wq": np.ascontiguousarray(W_qkv[:, 0 * D:1 * D][:, cs]),
            "wk": np.ascontiguousarray(W_qkv[:, 1 * D:2 * D][:, cs]),
            "wv": np.ascontiguousarray(W_qkv[:, 2 * D:3 * D][:, cs]),
            "wo": np.ascontiguousarray(W_out[cs, :]),
            "ones": np.ones((128, 64), dtype=np.float32),
        })

    nc = _build_program()
    res = run_bass_kernel_spmd(nc, in_maps, list(range(NCORES)), trace=TRACE)
    LAST_EXEC_NS = res.exec_time_ns
    LAST_RESULTS = res

    out = np.zeros((B, NQC, QC // KT, D // QC, 128, QC), dtype=np.float64)
    for c in range(NCORES):
        out += res.results[c]["out"].astype(np.float64)
    # [b, qc, ts, nn2, r, cc] -> [b, (qc ts r), (nn2 cc)]
    out = out.transpose(0, 1, 2, 4, 3, 5).reshape(B, T, D)
    return out.astype(np.float32)



# revision 10
# speedup vs baseline: 1.2345x; 1.2345x over previous
"""Causal multi-head attention (B=4, T=2048, D=1024, H=16) on 8 TRN2 NeuronCores.

Sharding: tensor-parallel over heads. Each core owns 2 heads (a contiguous
128-column block of each of W_q / W_k / W_v and a 128-row block of W_out).
Every core consumes the full (transposed) activation matrix xT and produces a
partial output [B*T, D]; the host sums the 8 partials (the "all-reduce").

All matmul operands are bf16 (fp32r runs as a 3-pass fp32_mode=HIGH emulation
on the PE - 3x slower; bf16 is 1 cycle/row and enables fast weight load).
PSUM accumulation stays fp32.

Per-core device pipeline:
  phase A (per batch b, per 512-wide chunk tcq):
      QT2[128,512] = Wq2h.T @ xT, KT2 likewise (accumulated over 8 dc chunks),
      V directly in [t, hd] layout: vv[128t,128hd] = xts_slice.T @ Wv2h
      (x as the stationary operand - no PE transpose needed), copied into
      vaug[128, kt, (64|1)x2] with a ones column per head (bf16).
  phase B (per b, per 512-wide query chunk qc): for each 128-wide key tile kt:
      ST_h[k,q] = KT2_h.T @ QT2_h (scores, transposed layout, PSUM fp32),
      expST = exp(ST/8) on ACT -> bf16 SBUF (straight-through softmax),
      causal mask applied POST-exp on the diagonal [128,128] block via
      gpsimd.affine_select (fill=0),
      OT_h[65,512] += Vaug_h.T @ expST (row 64 accumulates the denominator).
  phase C (per b, qc): recip = reciprocal_approx_fast(OT[64,:]) (one DVE op),
      gpsimd.partition_broadcast to 64 partitions, OTn = OT[0:64]*recip (bf16),
      out[t,:] partial = OTn.T @ W_out2h, staged to one [128,4096] bf16 SBUF
      tile, single 1MB DMA per (b,qc) to DRAM.
"""

import sys
import os

if "/opt/trn_rl_repo" not in sys.path:
    sys.path.insert(0, "/opt/trn_rl_repo")

import numpy as np

B, T, D, H, HD = 4, 2048, 1024, 16, 64
NCORES = 8
HPC = H // NCORES          # heads per core = 2
WBLK = HPC * HD            # 128: per-core head-block width
QC = 512                   # query chunk (matmul moving dim)
NQC = T // QC              # 4
KT = 128                   # key tile
NKT = T // KT              # 16
DC = 128                   # contraction chunk of D
NDC = D // DC              # 8

TRACE = False              # test.py sets kernel.TRACE = True for profiling
LAST_EXEC_NS = None
LAST_RESULTS = None

_MAX_WAITS = 1
_USE_RECIP_APPROX = False


def _make_tc_class():
    """TileContext patched for this container's walrus build, which rejects
    instructions carrying more than one sync-wait command (CTRL Drain,
    S3_LW ldweights, ...). Excess waits are hoisted onto freshly inserted
    same-engine NOPs placed immediately before the instruction (engine
    queues are in-order, so semantics are preserved)."""
    import concourse.tile as tile
    import concourse.mybir as mybir
    from concourse.vector_clock import VectorClock, ScopedClock

    class TC(tile.TileContext):
        def _drain_and_barrier(self, tick_clock, wait_clock):
            g = tick_clock.global_clock
            n = len(g)
            for proc in range(n):
                t = g[proc]
                if t > 0:
                    nop = self.nc.sync.nop(nofuse=True)
                    vc = VectorClock([0] * n)
                    vc.require_at_least(proc, t)
                    wait_clock.add_sem_waits(nop.ins, ScopedClock({None: vc}))
            self.nc.sync.drain()
            self.nc.all_engine_barrier()
            popped = self.nc._tile_sem_poison_stack.pop()
            assert popped is self._sem_poison
            self.nc.clear_and_free_semaphores(list(self.sems.allocated().values()))
            self.nc.all_engine_barrier()

        def _lower_ordered_insts(self, ordered):
            for bb_name in list(ordered.keys()):
                insts = ordered[bb_name]
                new_insts = []
                for inst in insts:
                    si = inst.sync_info
                    ow = list(si.on_wait) if si is not None and si.on_wait else []
                    if len(ow) > _MAX_WAITS:
                        keep = ow[:_MAX_WAITS]
                        extra = ow[_MAX_WAITS:]
                        for w in extra:
                            nop = mybir.InstNoOp(
                                name=f"WSPL-{self.nc.next_id()}", ins=[], outs=[]
                            )
                            nop.engine = inst.engine
                            nop.bass_nofuse = True
                            nop.sync_info = mybir.SyncInfo(on_wait=[w], on_update=[])
                            new_insts.append(nop)
                        inst.sync_info = mybir.SyncInfo(
                            on_wait=keep,
                            on_update=list(si.on_update) if si.on_update else [],
                        )
                    new_insts.append(inst)
                ordered[bb_name] = new_insts
            return super()._lower_ordered_insts(ordered)

    return TC


def _install_ntff_hook():
    """Provide antenv.axon_hooks (absent from the container's antenv stub) so
    run_bass_kernel_spmd(trace=True) can capture NTFF profiles."""
    import types
    import antenv

    if "antenv.axon_hooks" in sys.modules:
        return
    mod = types.ModuleType("antenv.axon_hooks")
    mod._hook = None
    mod.set_axon_ntff_profile_hook = lambda h: setattr(mod, "_hook", h)
    mod.get_axon_ntff_profile_hook = lambda: mod._hook
    sys.modules["antenv.axon_hooks"] = mod
    antenv.axon_hooks = mod
    try:
        from trn_agent_boot.trn_boot import _ntff_profile_via_ctypes

        hook = _ntff_profile_via_ctypes("/opt/axon/libaxon_pjrt.so")
        if hook is not None:
            mod.set_axon_ntff_profile_hook(hook)
    except Exception as e:  # profiling is best-effort
        print("ntff hook install failed:", e)


def _build_program():
    import concourse.bass as bass
    from concourse import mybir

    TC = _make_tc_class()
    f32 = mybir.dt.float32
    bf16 = mybir.dt.bfloat16
    Exp = mybir.ActivationFunctionType.Exp

    nc = bass.Bass("TRN2", target_bir_lowering=False, debug=False, num_devices=NCORES)
    # host-tiled xT: [b, tcq, 128, dc, 512] bf16, each [128, 4096] block contiguous
    xt_d = nc.dram_tensor("xt", [B, NQC, DC, NDC, QC], bf16, kind="ExternalInput")
    wq_d = nc.dram_tensor("wq", [D, WBLK], bf16, kind="ExternalInput")
    wk_d = nc.dram_tensor("wk", [D, WBLK], bf16, kind="ExternalInput")
    wv_d = nc.dram_tensor("wv", [D, WBLK], bf16, kind="ExternalInput")
    wo_d = nc.dram_tensor("wo", [WBLK, D], bf16, kind="ExternalInput")
    # output: [b, qc, 128, ts, nn2, 512] bf16, one [128,4096] DMA per (b,qc)
    out_d = nc.dram_tensor("out", [B, NQC, 128, QC // KT, D // QC, QC], bf16,
                           kind="ExternalOutput")

    with TC(nc, num_cores=NCORES) as tc:
        from contextlib import ExitStack

        with ExitStack() as ctx:
            constp = ctx.enter_context(tc.tile_pool(name="constp", bufs=1))
            wp = ctx.enter_context(tc.tile_pool(name="wp", bufs=1))
            xtp = ctx.enter_context(tc.tile_pool(name="xtp", bufs=3))
            qkp = ctx.enter_context(tc.tile_pool(name="qkp", bufs=2))
            vap = ctx.enter_context(tc.tile_pool(name="vap", bufs=2))
            expp = ctx.enter_context(tc.tile_pool(name="expp", bufs=3))
            otnp = ctx.enter_context(tc.tile_pool(name="otnp", bufs=2))
            outsbp = ctx.enter_context(tc.tile_pool(name="outsbp", bufs=2))
            rcpp = ctx.enter_context(tc.tile_pool(name="rcpp", bufs=2))
            bcp = ctx.enter_context(tc.tile_pool(name="bcp", bufs=2))
            # PSUM: mix(3) + st(3) + ot0(1) + ot1(1) = 8 banks
            stp = ctx.enter_context(tc.tile_pool(name="stp", bufs=3, space="PSUM"))
            mixp = ctx.enter_context(tc.tile_pool(name="mixp", bufs=3, space="PSUM"))
            otp = ctx.enter_context(tc.tile_pool(name="otp", bufs=1, space="PSUM"))

            # ---- one-time constants / weights ----
            ones64 = constp.tile([1, 64], f32)
            nc.gpsimd.memset(ones64[:], 1.0)
            wq_t = wp.tile([DC, NDC, WBLK], bf16)
            nc.gpsimd.dma_start(wq_t[:], wq_d.rearrange("(c p) m -> p c m", p=DC))
            wk_t = wp.tile([DC, NDC, WBLK], bf16)
            nc.gpsimd.dma_start(wk_t[:], wk_d.rearrange("(c p) m -> p c m", p=DC))
            wv_t = wp.tile([DC, NDC, WBLK], bf16)
            nc.gpsimd.dma_start(wv_t[:], wv_d.rearrange("(c p) m -> p c m", p=DC))
            wo_t = wp.tile([WBLK, D], bf16)
            nc.gpsimd.dma_start(wo_t[:], wo_d[:, :])

            for b in range(B):
                # ---------------- phase A: QKV projection ----------------
                qt2 = qkp.tile([128, T], bf16, tag="qt2", name=f"qt2_b{b}")
                kt2 = qkp.tile([128, T], bf16, tag="kt2", name=f"kt2_b{b}")
                # vaug[:, kt, h, 0:65] = [V_h | 1]
                vaug = vap.tile([128, NKT, 2, 65], bf16, tag="vaug", name=f"vaug_b{b}")
                nc.gpsimd.memset(vaug[:, :, :, 64], 1.0)

                for tcq in range(NQC):
                    xts = xtp.tile([DC, NDC, QC], bf16, tag="xt",
                                   name=f"xt_b{b}t{tcq}")
                    nc.gpsimd.dma_start(xts[:], xt_d[b, tcq])
                    accq = mixp.tile([128, QC], f32, tag="mix", name=f"accq_b{b}t{tcq}")
                    acck = mixp.tile([128, QC], f32, tag="mix", name=f"acck_b{b}t{tcq}")
                    for dc in range(NDC):
                        st, sp = (dc == 0), (dc == NDC - 1)
                        nc.tensor.matmul(
                            accq[:], wq_t[:, dc, :], xts[:, dc, :],
                            start=st, stop=sp, skip_group_check=True,
                        )
                        nc.tensor.matmul(
                            acck[:], wk_t[:, dc, :], xts[:, dc, :],
                            start=st, stop=sp, skip_group_check=True,
                        )
                    nc.vector.tensor_copy(qt2[:, tcq * QC:(tcq + 1) * QC], accq[:])
                    nc.vector.tensor_copy(kt2[:, tcq * QC:(tcq + 1) * QC], acck[:])
                    # V directly in [t, hd] layout: x-slice is the stationary
                    for ttl in range(QC // KT):
                        vv = mixp.tile([128, 2, 64], f32, tag="mix",
                                       name=f"vv_b{b}t{tcq}l{ttl}")
                        for dc in range(NDC):
                            nc.tensor.matmul(
                                vv[:, :, :],
                                xts[:, dc, ttl * KT:(ttl + 1) * KT],
                                wv_t[:, dc, :],
                                start=(dc == 0), stop=(dc == NDC - 1),
                                skip_group_check=True,
                            )
                        kt_g = tcq * (QC // KT) + ttl
                        # one strided copy: [128, 2, 64] -> vaug cols {0:64, 65:129}
                        nc.vector.tensor_copy(
                            vaug[:, kt_g, :, 0:64], vv[:, :, :]
                        )

                # ------------- phase B + C: attention per query chunk -------------
                for qc in range(NQC):
                    nkt = (qc + 1) * (QC // KT)
                    ot0 = otp.tile([65, QC], f32, tag="ot0", name=f"ot0_b{b}q{qc}")
                    ot1 = otp.tile([65, QC], f32, tag="ot1", name=f"ot1_b{b}q{qc}")
                    pend = None  # (expst0, expst1, kt, w0) awaiting AV matmul
                    for kt in range(nkt):
                        ksl = slice(kt * KT, (kt + 1) * KT)
                        # diagonal tiles: columns left of q = kt*128 are fully
                        # masked - skip them in scores/exp/AV entirely
                        j = kt - qc * (QC // KT)
                        w0 = KT * j if j > 0 else 0
                        csl = slice(w0, QC)
                        qsl_w = slice(qc * QC + w0, (qc + 1) * QC)
                        st0 = stp.tile([128, QC], f32, tag="st", name=f"st0_b{b}q{qc}k{kt}")
                        nc.tensor.matmul(
                            st0[:, csl], kt2[0:64, ksl], qt2[0:64, qsl_w],
                            start=True, stop=True, skip_group_check=True,
                        )
                        st1 = stp.tile([128, QC], f32, tag="st", name=f"st1_b{b}q{qc}k{kt}")
                        nc.tensor.matmul(
                            st1[:, csl], kt2[64:128, ksl], qt2[64:128, qsl_w],
                            start=True, stop=True, skip_group_check=True,
                        )
                        e0 = expp.tile([128, QC], bf16, tag="e0", name=f"e0_b{b}q{qc}k{kt}")
                        nc.scalar.activation(e0[:, csl], st0[:, csl], Exp, scale=0.125)
                        e1 = expp.tile([128, QC], bf16, tag="e1", name=f"e1_b{b}q{qc}k{kt}")
                        nc.scalar.activation(e1[:, csl], st1[:, csl], Exp, scale=0.125)
                        if 0 <= j < 4:
                            # zero the upper triangle of the diagonal block
                            # (post-exp, on SBUF bf16 - GPSIMD has no PSUM port)
                            for e in (e0, e1):
                                nc.gpsimd.affine_select(
                                    out=e[:, w0:w0 + KT],
                                    in_=e[:, w0:w0 + KT],
                                    compare_op=mybir.AluOpType.is_ge,
                                    fill=0.0,
                                    base=0,
                                    pattern=[[1, KT]],
                                    channel_multiplier=-1,
                                )
                        if pend is not None:
                            p0, p1, pk, pw = pend
                            nc.tensor.matmul(
                                ot0[0:65, pw:QC], vaug[:, pk, 0, :], p0[:, pw:QC],
                                start=(pk == 0), stop=False, skip_group_check=True,
                            )
                            nc.tensor.matmul(
                                ot1[0:65, pw:QC], vaug[:, pk, 1, :], p1[:, pw:QC],
                                start=(pk == 0), stop=False, skip_group_check=True,
                            )
                        pend = (e0, e1, kt, w0)
                    p0, p1, pk, pw = pend
                    nc.tensor.matmul(
                        ot0[0:65, pw:QC], vaug[:, pk, 0, :], p0[:, pw:QC],
                        start=(pk == 0), stop=True, skip_group_check=True,
                    )
                    nc.tensor.matmul(
                        ot1[0:65, pw:QC], vaug[:, pk, 1, :], p1[:, pw:QC],
                        start=(pk == 0), stop=True, skip_group_check=True,
                    )

                    # normalize: otn[h*64:(h+1)*64, :] = ot_h[0:64] * (1/denom_h)
                    # (reciprocal on the raw [1,512] denominator row via the
                    # single-op approx, then broadcast across 64 partitions
                    # with a rank-1 fp32 PE matmul)
                    otn = otnp.tile([128, QC], bf16, tag="otn", name=f"otn_b{b}q{qc}")
                    for h, ot in ((0, ot0), (1, ot1)):
                        den = rcpp.tile([1, QC], f32, tag=f"r{h}",
                                        name=f"den_b{b}q{qc}h{h}")
                        if _USE_RECIP_APPROX:
                            nc.vector.reciprocal_approx_fast(den[:], ot[64:65, :])
                        else:
                            nc.vector.reciprocal(den[:], ot[64:65, :])
                        bc = stp.tile([64, QC], f32, tag="st",
                                      name=f"bc_b{b}q{qc}h{h}")
                        nc.tensor.matmul(
                            bc[:], ones64[:], den[:],
                            start=True, stop=True, skip_group_check=True,
                        )
                        bcs = bcp.tile([64, QC], f32, tag=f"bc{h}",
                                       name=f"bcs_b{b}q{qc}h{h}")
                        nc.vector.tensor_copy(bcs[:], bc[:])
                        nc.vector.tensor_mul(
                            otn[h * 64:(h + 1) * 64, :], ot[0:64, :], bcs[:]
                        )

                    # out projection: out[t, :] += OTn.T @ W_out2h
                    outt = outsbp.tile([128, QC // KT, D // QC, QC], bf16,
                                       tag="osb", name=f"osb_b{b}q{qc}")
                    for ts in range(QC // KT):
                        for nn2 in range(D // QC):
                            ops = stp.tile(
                                [128, QC], f32, tag="st", name=f"ops_b{b}q{qc}s{ts}n{nn2}"
                            )
                            nc.tensor.matmul(
                                ops[:],
                                otn[:, ts * 128:(ts + 1) * 128],
                                wo_t[:, nn2 * QC:(nn2 + 1) * QC],
                                start=True, stop=True, skip_group_check=True,
                            )
                            nc.vector.tensor_copy(outt[:, ts, nn2, :], ops[:])
                    nc.sync.dma_start(out_d[b, qc], outt[:])
    return nc


def kernel(x, W_qkv, W_out):
    global LAST_EXEC_NS, LAST_RESULTS
    from concourse.bass_utils import run_bass_kernel_spmd
    import ml_dtypes

    if TRACE:
        _install_ntff_hook()

    bf = ml_dtypes.bfloat16
    x = np.ascontiguousarray(x, dtype=np.float32)
    W_qkv = np.ascontiguousarray(W_qkv, dtype=np.float32)
    W_out = np.ascontiguousarray(W_out, dtype=np.float32)

    xT = x.transpose(2, 0, 1).reshape(D, B * T)
    # tile to [b, tcq, 128, dc, 512] bf16 so each device DMA is one 1MB block
    xtt = np.ascontiguousarray(
        xT.reshape(NDC, DC, B, NQC, QC).transpose(2, 3, 1, 0, 4).astype(bf)
    )
    in_maps = []
    for c in range(NCORES):
        cs = slice(c * WBLK, (c + 1) * WBLK)
        in_maps.append({
            "xt": xtt,
            "wq": np.ascontiguousarray(W_qkv[:, 0 * D:1 * D][:, cs].astype(bf)),
            "wk": np.ascontiguousarray(W_qkv[:, 1 * D:2 * D][:, cs].astype(bf)),
            "wv": np.ascontiguousarray(W_qkv[:, 2 * D:3 * D][:, cs].astype(bf)),
            "wo": np.ascontiguousarray(W_out[cs, :].astype(bf)),
        })

    nc = _build_program()
    res = run_bass_kernel_spmd(nc, in_maps, list(range(NCORES)), trace=TRACE)
    LAST_EXEC_NS = res.exec_time_ns
    LAST_RESULTS = res

    out = np.zeros((B, NQC, 128, QC // KT, D // QC, QC), dtype=np.float32)
    for c in range(NCORES):
        out += res.results[c]["out"].astype(np.float32)
    # [b, qc, p, ts, nn2, cc] -> [b, (qc ts p), (nn2 cc)]
    out = out.transpose(0, 1, 3, 2, 4, 5).reshape(B, T, D)
    return np.ascontiguousarray(out)
